# revision 23
# baseline (speedup 1.0000x reference)
"""GPT forward pass on 8 Trainium2 NeuronCores, optimized for wire traffic.

DP2 x TP4 compute: cores 0-3 batch 0, cores 4-7 batch 1. Within a group:
heads 16->4/core, FFN 4096->1024/core, vocab 50257->12800/core (padded).
The axon tunnel to the devices is ~40 MB/s, so wall time is dominated by
host<->device bytes, not compute. Wire-traffic reductions vs the plain
DP2xTP4 layout:
  - Trunk weights are 10-bit quantized (per-input-channel f32 scales,
    offset-binary hi-byte plane + 2-bit los packed 4/byte), unpacked and
    dequantized to bf16 on device with vector bit ops + fused activation
    scale/bias. Weight quant error adds ~7e-3 to the logits.
  - No weight is duplicated across the two DP groups: each core uploads
    half of its TP-rank's weights and a pair AllGather ({r, r+4})
    reconstructs the full shard on device.
  - lm_head weights are int8 with per-(input-channel, 512-vocab-block)
    scales, also pair-split + AllGathered, dequantized to bf16 on device.
  - x0 embeddings are bf16 quarter-shards AllGathered within each DP group.
  - The causal mask is generated on device (gpsimd affine_select).
  - Logits are returned as int8 with per-(token, 512-vocab-block) f32
    scales (the f32->int8 copy rounds to nearest on HW) and dequantized
    on host. This also halves the donated zero output buffer upload.
Compute structure is unchanged from the bf16 baseline: residual stream
transposed [D, S] fp32 in SBUF; bf16 matmuls, fp32 PSUM; LN scale/bias +
attention scale folded into weights host-side; softmax on transposed
scores without max-subtraction; two bf16 AllReduces per layer over
4-rank groups.
"""

import contextlib

import numpy as np
import ml_dtypes

import concourse.bacc as bacc
import concourse.tile as tile
from concourse import mybir
from concourse.bass_utils import run_bass_kernel_spmd

BF = mybir.dt.bfloat16
F32 = mybir.dt.float32
I8 = mybir.dt.int8
U8 = mybir.dt.uint8
NPBF = ml_dtypes.bfloat16
AF = mybir.ActivationFunctionType
ALU = mybir.AluOpType

B, S, D, H, L, V = 2, 1024, 1024, 16, 8, 50257
HD, FF, EPS = 64, 4096, 1e-5
TP, HPC, FFC, VP, NC = 4, 4, 1024, 12800, 8
KC, TT, TS = 8, 2, 512
NT = VP // TS  # 25 vocab tiles per core

PAIRS = [[0, 4], [1, 5], [2, 6], [3, 7]]
QUADS = [[0, 1, 2, 3], [4, 5, 6, 7]]


def _pmajor(wt):
    """[K_contract, N] -> [128, K//128, N] partition-major."""
    k, n = wt.shape
    return np.ascontiguousarray(wt.reshape(k // 128, 128, n).transpose(1, 0, 2))


def _pack10(wt):
    """[K, M] f32 -> ([128, K//128, M + M//4] u8 packed uint10, [128, K//128]
    f32 scales). Values stored offset-binary: hi byte plane then lo 2-bit
    pairs packed 4/byte; w = (4*hi + lo - 512) * s with s per input chan."""
    k, m = wt.shape
    s = np.maximum(np.abs(wt).max(1) / 511.0, 1e-30)
    q = np.clip(np.rint(wt / s[:, None]), -511, 511).astype(np.int32)
    qu = (q + 512).astype(np.uint16)
    hi = (qu >> 2).astype(np.uint8)
    lo = (qu & 3).astype(np.uint8)
    lb = (lo[:, 0::4] | (lo[:, 1::4] << 2) | (lo[:, 2::4] << 4)
          | (lo[:, 3::4] << 6))
    pk = np.concatenate([hi, lb], 1)
    return _pmajor(pk), s.reshape(k // 128, 128).T.astype(np.float32)


def build_nc(n_layers=L):
    nc = bacc.Bacc("TRN2", target_bir_lowering=False, debug=False,
                   num_devices=NC)
    Lc = n_layers

    # ---- params: halves of pair-shared tensors, full small tensors ----
    # Trunk weights are 10-bit packed: per input channel, uint10 offset-
    # binary values as [hi byte | packed lo 2-bit pairs] along the free dim.
    x0q_d = nc.declare_dram_parameter("x0q", [32, KC, S], BF, isOutput=False)
    qkvwh_d = nc.declare_dram_parameter("qkvwh", [Lc * 64, KC, 960], U8, isOutput=False)
    projwh_d = nc.declare_dram_parameter("projwh", [Lc * 64, 2, 1280], U8, isOutput=False)
    fc1wh_d = nc.declare_dram_parameter("fc1wh", [Lc * 64, KC, 1280], U8, isOutput=False)
    fc2wh_d = nc.declare_dram_parameter("fc2wh", [Lc * 64, KC, 1280], U8, isOutput=False)
    qsc_d = nc.declare_dram_parameter("qsc", [128, Lc * KC], F32, isOutput=False)
    psc_d = nc.declare_dram_parameter("psc", [128, Lc * 2], F32, isOutput=False)
    f1sc_d = nc.declare_dram_parameter("f1sc", [128, Lc * KC], F32, isOutput=False)
    f2sc_d = nc.declare_dram_parameter("f2sc", [128, Lc * KC], F32, isOutput=False)
    headw8h_d = nc.declare_dram_parameter("headw8h", [64, KC, VP], I8, isOutput=False)
    headsc_d = nc.declare_dram_parameter("headsc", [128, KC * NT], F32, isOutput=False)
    qkvb_d = nc.declare_dram_parameter("qkvb", [Lc, 128, 6], F32, isOutput=False)
    projb_d = nc.declare_dram_parameter("projb", [Lc, 128, KC], F32, isOutput=False)
    fc1b_d = nc.declare_dram_parameter("fc1b", [Lc, 128, KC], F32, isOutput=False)
    fc2b_d = nc.declare_dram_parameter("fc2b", [Lc, 128, KC], F32, isOutput=False)
    headb_d = nc.declare_dram_parameter("headb", [1, VP], BF, isOutput=False)
    id_d = nc.declare_dram_parameter("id64", [64, 64], BF, isOutput=False)
    out_d = nc.declare_dram_parameter("out", [S, VP], I8, isOutput=True)
    oscale_d = nc.declare_dram_parameter("oscale", [NT, S], F32, isOutput=True)

    # ---- internal DRAM: staged halves (collectives can't read IO) + fulls
    x0q_i = nc.dram_tensor("x0q_i", [32, KC, S], BF)
    qkvwh_i = nc.dram_tensor("qkvwh_i", [Lc * 64, KC, 960], U8)
    projwh_i = nc.dram_tensor("projwh_i", [Lc * 64, 2, 1280], U8)
    fc1wh_i = nc.dram_tensor("fc1wh_i", [Lc * 64, KC, 1280], U8)
    fc2wh_i = nc.dram_tensor("fc2wh_i", [Lc * 64, KC, 1280], U8)
    headw8h_i = nc.dram_tensor("headw8h_i", [64, KC, VP], I8)
    x0_f = nc.dram_tensor("x0_f", [128, KC, S], BF)
    qkvw_f = nc.dram_tensor("qkvw_f", [Lc * 128, KC, 960], U8)
    projw_f = nc.dram_tensor("projw_f", [Lc * 128, 2, 1280], U8)
    fc1w_f = nc.dram_tensor("fc1w_f", [Lc * 128, KC, 1280], U8)
    fc2w_f = nc.dram_tensor("fc2w_f", [Lc * 128, KC, 1280], U8)
    headw8_f = nc.dram_tensor("headw8_f", [128, KC, VP], I8)

    ar_in = [nc.dram_tensor(f"arin{i}", [128, KC, S], BF) for i in range(2 * Lc)]
    ar_out = [nc.dram_tensor(f"arout{i}", [128, KC, S], BF) for i in range(2 * Lc)]

    with tile.TileContext(nc, num_cores=NC) as tc, contextlib.ExitStack() as ctx:
        # ---- reconstruct pair/quad-shared tensors on device ----
        for src, dst in ((x0q_d, x0q_i), (qkvwh_d, qkvwh_i),
                         (projwh_d, projwh_i), (fc1wh_d, fc1wh_i),
                         (fc2wh_d, fc2wh_i), (headw8h_d, headw8h_i)):
            nc.sync.dma_start(out=dst[:], in_=src[:])
        nc.gpsimd.collective_compute(
            "AllGather", ALU.bypass, ins=[x0q_i.ap().opt()],
            outs=[x0_f.ap().opt()], replica_groups=QUADS)
        for src, dst in ((qkvwh_i, qkvw_f), (projwh_i, projw_f),
                         (fc1wh_i, fc1w_f), (fc2wh_i, fc2w_f),
                         (headw8h_i, headw8_f)):
            nc.gpsimd.collective_compute(
                "AllGather", ALU.bypass, ins=[src.ap().opt()],
                outs=[dst.ap().opt()], replica_groups=PAIRS)

        # ---- persistent pools (LN machinery, residual, outputs) ----
        consts = ctx.enter_context(tc.tile_pool(name="consts", bufs=1))
        xpool = ctx.enter_context(tc.tile_pool(name="x", bufs=1))
        zpool = ctx.enter_context(tc.tile_pool(name="z", bufs=1))
        rows = ctx.enter_context(tc.tile_pool(name="rows", bufs=2))
        qrows = ctx.enter_context(tc.tile_pool(name="qrows", bufs=2))
        bcast = ctx.enter_context(tc.tile_pool(name="bcast", bufs=2))
        sq_p = ctx.enter_context(tc.tile_pool(name="sq", bufs=2))
        outp = ctx.enter_context(tc.tile_pool(name="outs", bufs=2))
        ps_st = ctx.enter_context(tc.tile_pool(name="psst", bufs=2, space="PSUM"))
        ps_bc = ctx.enter_context(tc.tile_pool(name="psbc", bufs=1, space="PSUM"))

        mask_sb = consts.tile([128, 4, TS], BF, tag="mask")
        for j in range(4):
            # mask[p, j, s] = 1.0 if s >= j*128 + p else 0.0
            nc.gpsimd.memset(mask_sb[:, j, :], 1.0)
            nc.gpsimd.affine_select(
                out=mask_sb[:, j, :], in_=mask_sb[:, j, :],
                compare_op=ALU.is_ge, fill=0.0, base=-j * 128,
                pattern=[[1, TS]], channel_multiplier=-1)
        idt = consts.tile([128, 64], BF, tag="idt")
        nc.sync.dma_start(out=idt[0:64, :], in_=id_d[:])
        nc.sync.dma_start(out=idt[64:128, :], in_=id_d[:])
        ones = consts.tile([128, 128], BF, tag="ones")
        nc.vector.memset(ones[:], 1.0)
        hsc = consts.tile([128, KC * NT], F32, tag="hsc")
        nc.sync.dma_start(out=hsc[:], in_=headsc_d[:])
        # trunk weight scales + their -2048*s biases for 12-bit unpack
        wsc, wnb = {}, {}
        for nm, src, ncol in (("q", qsc_d, Lc * KC), ("p", psc_d, Lc * 2),
                              ("f1", f1sc_d, Lc * KC), ("f2", f2sc_d, Lc * KC)):
            sct = consts.tile([128, ncol], F32, tag=f"sc_{nm}")
            nc.sync.dma_start(out=sct[:], in_=src[:])
            nbt = consts.tile([128, ncol], F32, tag=f"nb_{nm}")
            nc.vector.tensor_scalar_mul(nbt[:], sct[:], -512.0)
            wsc[nm], wnb[nm] = sct, nbt

        x_sb = xpool.tile([128, KC, S], F32, tag="x")
        zx = zpool.tile([128, KC, S], BF, tag="z")  # bf16 x, normalized in place
        # x0 arrives bf16 via the quad AllGather; stage through zx into f32.
        nc.sync.dma_start(out=zx[:], in_=x0_f[:])
        for c in range(KC):
            nc.scalar.activation(out=x_sb[:, c, :], in_=zx[:, c, :], func=AF.Copy)

        def layer_norm():
            for c in range(KC):
                nc.scalar.activation(out=zx[:, c, :], in_=x_sb[:, c, :],
                                     func=AF.Copy)
            mu_b, rs_b = [None] * TT, [None] * TT
            for t in range(TT):
                sl = slice(t * TS, (t + 1) * TS)
                ps_s = ps_st.tile([1, TS], F32, tag="st")
                ps_q = ps_st.tile([1, TS], F32, tag="st")
                for c in range(KC):
                    sq = sq_p.tile([128, TS], BF, tag="sq")
                    nc.vector.tensor_mul(sq[:], zx[:, c, sl], zx[:, c, sl])
                    nc.tensor.matmul(ps_s[:], ones[:, 0:1], zx[:, c, sl],
                                     start=(c == 0), stop=(c == KC - 1))
                    nc.tensor.matmul(ps_q[:], ones[:, 0:1], sq[:],
                                     start=(c == 0), stop=(c == KC - 1))
                mu = rows.tile([1, TS], F32, tag="mu")
                nc.vector.tensor_scalar_mul(mu[:], ps_s[:], 1.0 / D)
                ms = rows.tile([1, TS], F32, tag="ms")
                nc.vector.tensor_mul(ms[:], mu[:], mu[:])
                ve = rows.tile([1, TS], F32, tag="ve")
                nc.vector.tensor_scalar(ve[:], ps_q[:], 1.0 / D, EPS,
                                        op0=ALU.mult, op1=ALU.add)
                nc.vector.tensor_sub(ve[:], ve[:], ms[:])
                sd = rows.tile([1, TS], F32, tag="sd")
                nc.scalar.activation(out=sd[:], in_=ve[:], func=AF.Sqrt)
                rs = rows.tile([1, TS], BF, tag="rs")
                with nc.allow_low_precision(reason="bf16 rstd row"):
                    nc.vector.reciprocal(rs[:], sd[:])
                mubf = rows.tile([1, TS], BF, tag="mubf")
                nc.vector.tensor_copy(mubf[:], mu[:])
                ps_mb = ps_bc.tile([128, TS], F32, tag="bc")
                nc.tensor.matmul(ps_mb[:], ones[0:1, :], mubf[:])
                mb = bcast.tile([128, TS], BF, tag="mb")
                nc.scalar.activation(out=mb[:], in_=ps_mb[:], func=AF.Copy)
                ps_rb = ps_bc.tile([128, TS], F32, tag="bc")
                nc.tensor.matmul(ps_rb[:], ones[0:1, :], rs[:])
                rb = bcast.tile([128, TS], BF, tag="rb")
                nc.scalar.activation(out=rb[:], in_=ps_rb[:], func=AF.Copy)
                mu_b[t], rs_b[t] = mb, rb
            for c in range(KC):
                for t in range(TT):
                    sl = slice(t * TS, (t + 1) * TS)
                    nc.vector.tensor_sub(zx[:, c, sl], zx[:, c, sl], mu_b[t][:])
                    nc.vector.tensor_mul(zx[:, c, sl], zx[:, c, sl], rs_b[t][:])

        # ---- trunk ----
        with contextlib.ExitStack() as tctx:
            wq = tctx.enter_context(tc.tile_pool(name="wq", bufs=1))
            wp = tctx.enter_context(tc.tile_pool(name="wp", bufs=1))
            w1 = tctx.enter_context(tc.tile_pool(name="w1", bufs=1))
            w2 = tctx.enter_context(tc.tile_pool(name="w2", bufs=1))
            bpool = tctx.enter_context(tc.tile_pool(name="bias", bufs=2))
            qkvo = tctx.enter_context(tc.tile_pool(name="qkvo", bufs=1))
            probs = tctx.enter_context(tc.tile_pool(name="probs", bufs=1))
            attn = tctx.enter_context(tc.tile_pool(name="attn", bufs=2))
            apool = tctx.enter_context(tc.tile_pool(name="act", bufs=1))
            stage = tctx.enter_context(tc.tile_pool(name="stage", bufs=3))
            pkst = tctx.enter_context(tc.tile_pool(name="pkst", bufs=2))
            lost = tctx.enter_context(tc.tile_pool(name="lost", bufs=1))
            accst = tctx.enter_context(tc.tile_pool(name="accst", bufs=1))
            ps_mm = tctx.enter_context(
                tc.tile_pool(name="psmm", bufs=2, space="PSUM"))
            ps_sc = tctx.enter_context(
                tc.tile_pool(name="pssc", bufs=2, space="PSUM"))
            ps_ao = tctx.enter_context(
                tc.tile_pool(name="psao", bufs=1, space="PSUM"))

            def load_packed(dst_bf, pk_f, lsl, kc_n, m, nm, l):
                """10-bit unpack: dst = (4*hi + lo - 512) * s, per in-chan."""
                sct, nbt = wsc[nm], wnb[nm]
                q = m // 4
                for kc in range(kc_n):
                    co = l * kc_n + kc
                    pk = pkst.tile([128, 1280], U8, tag="pk")
                    nc.sync.dma_start(out=pk[:, 0:m + q],
                                      in_=pk_f[lsl, kc, :])
                    acc = accst.tile([128, m], F32, tag="acc")
                    nc.vector.tensor_scalar_mul(acc[:], pk[:, 0:m], 4.0)
                    for j in range(4):
                        lo = lost.tile([128, q], U8, tag=f"lo{j}")
                        if j == 0:
                            nc.vector.tensor_scalar(
                                lo[:], pk[:, m:m + q], 3, None,
                                op0=ALU.bitwise_and)
                        elif j == 3:
                            nc.vector.tensor_scalar(
                                lo[:], pk[:, m:m + q], 6, None,
                                op0=ALU.logical_shift_right)
                        else:
                            nc.vector.tensor_scalar(
                                lo[:], pk[:, m:m + q], 2 * j, 3,
                                op0=ALU.logical_shift_right,
                                op1=ALU.bitwise_and)
                        nc.vector.tensor_add(acc[:, j:m:4], acc[:, j:m:4],
                                             lo[:])
                    nc.scalar.activation(out=dst_bf[:, kc, :], in_=acc[:],
                                         func=AF.Identity,
                                         scale=sct[:, co:co + 1],
                                         bias=nbt[:, co:co + 1])

            def mm_block(wt, bias_tile, rhs_sb, n_out, kc_n, out_cb, func):
                for mt in range(n_out // 128):
                    msl = slice(mt * 128, (mt + 1) * 128)
                    for t in range(TT):
                        sl = slice(t * TS, (t + 1) * TS)
                        ps = ps_mm.tile([128, TS], F32, tag="mm")
                        for kc in range(kc_n):
                            nc.tensor.matmul(ps[:], wt[:, kc, msl],
                                             rhs_sb(kc, sl), start=(kc == 0),
                                             stop=(kc == kc_n - 1))
                        out_cb(mt, t, ps, bias_tile[:, mt:mt + 1], func)

            def evict(dst_ap):
                def cb(mt, t, ps, bias, func):
                    nc.scalar.activation(out=dst_ap(mt, t), in_=ps[:],
                                         func=func, bias=bias)
                return cb

            def evict_ar(ar_buf):
                def cb(mt, t, ps, bias, func):
                    st = stage.tile([128, TS], BF, tag="arst")
                    nc.scalar.activation(out=st[:], in_=ps[:], func=func,
                                         bias=bias)
                    nc.sync.dma_start(out=ar_buf[:, mt, t * TS:(t + 1) * TS],
                                      in_=st[:])
                return cb

            def allreduce_residual(li):
                nc.gpsimd.collective_compute(
                    "AllReduce", ALU.add,
                    ins=[ar_in[li].ap().opt()], outs=[ar_out[li].ap().opt()],
                    replica_groups=QUADS)
                for c in range(KC):
                    st = stage.tile([128, S], BF, tag="arld")
                    nc.sync.dma_start(out=st[:], in_=ar_out[li][:, c, :])
                    nc.vector.tensor_add(x_sb[:, c, :], x_sb[:, c, :], st[:])

            for l in range(Lc):
                lsl = slice(l * 128, (l + 1) * 128)
                layer_norm()
                qw = wq.tile([128, KC, 768], BF, tag="qkvw")
                load_packed(qw, qkvw_f, lsl, KC, 768, "q", l)
                qb = bpool.tile([128, 6], F32, tag="qkvb")
                nc.sync.dma_start(out=qb[:], in_=qkvb_d[l])
                qkv = qkvo.tile([128, 6, S], BF, tag="qkv")
                mm_block(qw, qb, lambda kc, sl: zx[:, kc, sl], 768, KC,
                         evict(lambda mt, t: qkv[:, mt, t * TS:(t + 1) * TS]),
                         AF.Identity)

                aon = attn.tile([128, 2, S], BF, tag="aon")
                for h in range(HPC):
                    hb = (h % 2) * 64
                    hsl = slice(hb, hb + 64)
                    vt = attn.tile([128, KC, 64], BF, tag="vt")
                    for kt in range(KC):
                        pvt = ps_mm.tile([128, 64], BF, tag="mm")
                        nc.tensor.transpose(
                            pvt[:], qkv[hsl, 4 + h // 2, kt * 128:(kt + 1) * 128],
                            idt[hsl, :])
                        nc.scalar.activation(out=vt[:, kt, :], in_=pvt[:],
                                             func=AF.Copy)
                    for t in range(TT):
                        sl = slice(t * TS, (t + 1) * TS)
                        nkt = 4 * (t + 1)
                        pb = probs.tile([128, KC, TS], BF, tag="probs")
                        ps_d = ps_st.tile([1, TS], F32, tag="st")
                        for kt in range(nkt):
                            psc = ps_sc.tile([128, TS], F32, tag="sc")
                            nc.tensor.matmul(
                                psc[:], qkv[hsl, 2 + h // 2,
                                            kt * 128:(kt + 1) * 128],
                                qkv[hsl, h // 2, sl])
                            nc.scalar.activation(out=pb[:, kt, :], in_=psc[:],
                                                 func=AF.Exp)
                            moff = kt - t * 4
                            if moff >= 0:
                                nc.vector.tensor_mul(
                                    pb[:, kt, :], pb[:, kt, :],
                                    mask_sb[:, moff, :])
                            nc.tensor.matmul(ps_d[:], ones[:, 0:1],
                                             pb[:, kt, :], start=(kt == 0),
                                             stop=(kt == nkt - 1))
                        rr = rows.tile([1, TS], BF, tag="rr")
                        with nc.allow_low_precision(reason="bf16 softmax recip"):
                            nc.vector.reciprocal(rr[:], ps_d[:])
                        ps_rb = ps_bc.tile([128, TS], F32, tag="bc")
                        nc.tensor.matmul(ps_rb[hsl, :], ones[0:1, hsl], rr[:])
                        rb = bcast.tile([128, TS], BF, tag="arb")
                        nc.scalar.activation(out=rb[hsl, :], in_=ps_rb[hsl, :],
                                             func=AF.Copy)
                        pao = ps_ao.tile([128, TS], F32, tag="ao")
                        for kt in range(nkt):
                            nc.tensor.matmul(pao[hsl, :], vt[:, kt, :],
                                             pb[:, kt, :], start=(kt == 0),
                                             stop=(kt == nkt - 1))
                        nc.vector.tensor_mul(aon[hsl, h // 2, sl],
                                             pao[hsl, :], rb[hsl, :])

                pw = wp.tile([128, 2, D], BF, tag="projw")
                load_packed(pw, projw_f, lsl, 2, D, "p", l)
                pbias = bpool.tile([128, KC], F32, tag="projb")
                nc.sync.dma_start(out=pbias[:], in_=projb_d[l])
                mm_block(pw, pbias, lambda kc, sl: aon[:, kc, sl], D, 2,
                         evict_ar(ar_in[2 * l]), AF.Identity)
                allreduce_residual(2 * l)

                layer_norm()
                w1t = w1.tile([128, KC, FFC], BF, tag="fc1w")
                load_packed(w1t, fc1w_f, lsl, KC, FFC, "f1", l)
                b1 = bpool.tile([128, KC], F32, tag="fc1b")
                nc.sync.dma_start(out=b1[:], in_=fc1b_d[l])
                a_sb = apool.tile([128, KC, S], BF, tag="a")
                mm_block(w1t, b1, lambda kc, sl: zx[:, kc, sl], FFC, KC,
                         evict(lambda mt, t: a_sb[:, mt, t * TS:(t + 1) * TS]),
                         AF.Relu)
                w2t = w2.tile([128, KC, D], BF, tag="fc2w")
                load_packed(w2t, fc2w_f, lsl, KC, D, "f2", l)
                b2 = bpool.tile([128, KC], F32, tag="fc2b")
                nc.sync.dma_start(out=b2[:], in_=fc2b_d[l])
                mm_block(w2t, b2, lambda kc, sl: a_sb[:, kc, sl], D, KC,
                         evict_ar(ar_in[2 * l + 1]), AF.Identity)
                allreduce_residual(2 * l + 1)

            layer_norm()

        # ---- lm_head (trunk pools closed; zx holds final LN output) ----
        with contextlib.ExitStack() as lctx:
            hwp = lctx.enter_context(tc.tile_pool(name="hw", bufs=3))
            hw8p = lctx.enter_context(tc.tile_pool(name="hw8", bufs=2))
            ps_lm = lctx.enter_context(
                tc.tile_pool(name="pslm", bufs=3, space="PSUM"))
            hb_row = hwp.tile([1, VP], BF, tag="hbrow")
            nc.sync.dma_start(out=hb_row[:], in_=headb_d[:])
            hw_t = {}
            for nt in range(NT):
                t8 = hw8p.tile([128, KC, TS], I8, tag="hw8")
                nc.sync.dma_start(out=t8[:],
                                  in_=headw8_f[:, :, nt * TS:(nt + 1) * TS])
                t0 = hwp.tile([128, KC, TS], BF, tag="hw")
                for kc in range(KC):
                    nc.scalar.activation(
                        out=t0[:, kc, :], in_=t8[:, kc, :], func=AF.Copy,
                        scale=hsc[:, kc * NT + nt:kc * NT + nt + 1])
                hw_t[nt] = t0
                if nt % 2 == 1 or nt == NT - 1:
                    nts = [nt - 1, nt] if nt % 2 == 1 else [nt]
                    bsts = {}
                    for n2 in nts:
                        psb = ps_bc.tile([128, TS], F32, tag="bc")
                        nc.tensor.matmul(psb[:], ones[0:1, :],
                                         hb_row[:, n2 * TS:(n2 + 1) * TS])
                        bst = bcast.tile([128, TS], F32, tag="hb")
                        nc.scalar.activation(out=bst[:], in_=psb[:],
                                             func=AF.Copy)
                        bsts[n2] = bst
                    for mt in range(KC):
                        pss = {n2: ps_lm.tile([128, TS], F32, tag="lm",
                                              name=f"pslm{n2 % 2}")
                               for n2 in nts}
                        for kc in range(KC):
                            for n2 in nts:
                                nc.tensor.matmul(
                                    pss[n2][:],
                                    zx[:, kc, mt * 128:(mt + 1) * 128],
                                    hw_t[n2][:, kc, :], start=(kc == 0),
                                    stop=(kc == KC - 1))
                        for n2 in nts:
                            of = outp.tile([128, TS], F32, tag="of")
                            nc.vector.tensor_add(of[:], pss[n2][:],
                                                 bsts[n2][:])
                            mx = qrows.tile([128, 1], F32, tag="mx")
                            nc.vector.reduce_max(
                                out=mx[:], in_=of[:],
                                axis=mybir.AxisListType.X,
                                apply_absolute_value=True)
                            sc = qrows.tile([128, 1], F32, tag="sc")
                            nc.vector.tensor_scalar(
                                sc[:], mx[:], 1.0 / 126.0, 1e-30,
                                op0=ALU.mult, op1=ALU.max)
                            iv = qrows.tile([128, 1], F32, tag="iv")
                            nc.vector.reciprocal(iv[:], sc[:])
                            nc.vector.tensor_scalar_mul(of[:], of[:], iv[:])
                            qt = outp.tile([128, TS], I8, tag="qt")
                            nc.scalar.activation(out=qt[:], in_=of[:],
                                                 func=AF.Copy)
                            nc.sync.dma_start(
                                out=out_d[mt * 128:(mt + 1) * 128,
                                          n2 * TS:(n2 + 1) * TS], in_=qt[:])
                            nc.sync.dma_start(
                                out=oscale_d[n2, mt * 128:(mt + 1) * 128],
                                in_=sc[:])
                    for n2 in nts:
                        del hw_t[n2]
    nc.compile()
    return nc


def prep_inputs(inputs, n_layers=L):
    i = {k: np.asarray(v) for k, v in inputs.items()}
    idx, tok_emb, pos_emb = i["idx"], i["tok_emb"], i["pos_emb"]
    hw_pad = np.zeros((TP * VP, D), np.float32)
    hw_pad[:V] = i["head_w"] * i["lnf_s"][None, :]
    hb_pad = np.zeros((TP * VP,), np.float32)
    hb_pad[:V] = i["head_b"] + i["head_w"] @ i["lnf_b"]
    id64 = np.eye(64, dtype=NPBF)

    # per-group x0 quarter shards (bf16, partition-major flat quarters)
    x0q = {}
    for g in range(B):
        x0 = (tok_emb[idx[g]] + pos_emb).astype(np.float32)  # [S, D]
        pm = np.ascontiguousarray(
            x0.T.reshape(KC, 128, S).transpose(1, 0, 2)).astype(NPBF)
        x0q[g] = pm.reshape(4, 32, KC, S)

    # per-rank weights (computed once, split into pair halves)
    rank = []
    for r in range(TP):
        qkvw = np.empty((n_layers, 128, KC, 960), np.uint8)
        qsc = np.empty((n_layers, 128, KC), np.float32)
        qkvb = np.empty((n_layers, 128, 6), np.float32)
        projw = np.empty((n_layers, 128, 2, 1280), np.uint8)
        psc = np.empty((n_layers, 128, 2), np.float32)
        projb = np.empty((n_layers, 128, KC), np.float32)
        fc1w = np.empty((n_layers, 128, KC, 1280), np.uint8)
        f1sc = np.empty((n_layers, 128, KC), np.float32)
        fc1b = np.empty((n_layers, 128, KC), np.float32)
        fc2w = np.empty((n_layers, 128, KC, 1280), np.uint8)
        f2sc = np.empty((n_layers, 128, KC), np.float32)
        fc2b = np.empty((n_layers, 128, KC), np.float32)
        for l in range(n_layers):
            qw = i["qkv_w"][l]  # [3D, D]; row h*192 + {q:0,k:64,v:128} + hd
            blk = {"q": [], "k": [], "v": []}
            for j in range(HPC):
                h = r * HPC + j
                blk["q"].append(qw[h * 192:h * 192 + 64])
                blk["k"].append(qw[h * 192 + 64:h * 192 + 128])
                blk["v"].append(qw[h * 192 + 128:h * 192 + 192])
            W = np.concatenate(blk["q"] + blk["k"] + blk["v"], 0)  # [768, D]
            beff = W @ i["ln1_b"][l]
            Wp = W * i["ln1_s"][l][None, :]
            Wp[:256] *= HD ** -0.5
            beff[:256] *= HD ** -0.5
            qkvw[l], qsc[l] = _pack10(Wp.T)
            qkvb[l] = beff.reshape(6, 128).T
            projw[l], psc[l] = _pack10(
                i["proj_w"][l][:, r * 256:(r + 1) * 256].T)
            projb[l] = (i["proj_b"][l] / TP).reshape(KC, 128).T
            W1 = i["fc1_w"][l][r * FFC:(r + 1) * FFC]  # [FFC, D]
            fc1b[l] = (i["fc1_b"][l][r * FFC:(r + 1) * FFC]
                       + W1 @ i["ln2_b"][l]).reshape(KC, 128).T
            fc1w[l], f1sc[l] = _pack10((W1 * i["ln2_s"][l][None, :]).T)
            fc2w[l], f2sc[l] = _pack10(
                i["fc2_w"][l][:, r * FFC:(r + 1) * FFC].T)
            fc2b[l] = (i["fc2_b"][l] / TP).reshape(KC, 128).T

        # int8 head weights, scale per (input channel, 512-vocab block)
        WT = np.ascontiguousarray(hw_pad[r * VP:(r + 1) * VP].T)  # [D, VP]
        scs = np.maximum(
            np.abs(WT.reshape(D, NT, TS)).max(2) / 127.0, 1e-30)  # [D, NT]
        q8 = np.clip(np.rint(WT.reshape(D, NT, TS) / scs[:, :, None]),
                     -127, 127).astype(np.int8).reshape(D, VP)
        headw8 = _pmajor(q8)  # [128, KC, VP] int8
        headsc = np.ascontiguousarray(
            scs.reshape(KC, 128, NT).transpose(1, 0, 2)
        ).reshape(128, KC * NT).astype(np.float32)

        halves = {}
        for name, arr in (("qkvwh", qkvw), ("projwh", projw),
                          ("fc1wh", fc1w), ("fc2wh", fc2w)):
            flat = arr.reshape(n_layers * 128, *arr.shape[2:])
            halves[name] = (np.ascontiguousarray(flat[:n_layers * 64]),
                            np.ascontiguousarray(flat[n_layers * 64:]))
        halves["headw8h"] = (np.ascontiguousarray(headw8[:64]),
                             np.ascontiguousarray(headw8[64:]))

        def sc_cols(a):  # [Lc,128,g] -> [128, Lc*g] with column l*g + kc
            return np.ascontiguousarray(
                a.transpose(1, 0, 2).reshape(128, -1))
        rank.append(dict(
            halves=halves, headsc=headsc, qkvb=qkvb, projb=projb,
            fc1b=fc1b, fc2b=fc2b, qsc=sc_cols(qsc), psc=sc_cols(psc),
            f1sc=sc_cols(f1sc), f2sc=sc_cols(f2sc),
            headb=hb_pad[None, r * VP:(r + 1) * VP].astype(NPBF)))

    in_maps = []
    for core in range(NC):
        g, r = divmod(core, TP)
        rd = rank[r]
        m = {
            "x0q": np.ascontiguousarray(x0q[g][r]),
            "id64": id64,
            "headsc": rd["headsc"], "headb": rd["headb"],
            "qkvb": rd["qkvb"], "projb": rd["projb"],
            "fc1b": rd["fc1b"], "fc2b": rd["fc2b"],
            "qsc": rd["qsc"], "psc": rd["psc"],
            "f1sc": rd["f1sc"], "f2sc": rd["f2sc"],
        }
        for name in ("qkvwh", "projwh", "fc1wh", "fc2wh", "headw8h"):
            m[name] = rd["halves"][name][g]
        in_maps.append(m)
    return in_maps


_NC_CACHE = {}


def kernel(**inputs):
    if L not in _NC_CACHE:
        _NC_CACHE[L] = build_nc(L)
    nc = _NC_CACHE[L]
    in_maps = prep_inputs(inputs)
    res = run_bass_kernel_spmd(nc, in_maps, core_ids=list(range(NC)))
    return assemble_output(res)


def assemble_output(res):
    out = np.empty((B, S, V), np.float32)
    for g in range(B):
        parts = []
        for r in range(TP):
            rr = res.results[g * TP + r]
            q = rr["out"].astype(np.float32).reshape(S, NT, TS)
            q *= rr["oscale"].T[:, :, None]  # [S, NT, 1]
            parts.append(q.reshape(S, VP))
        out[g] = np.concatenate(parts, axis=1)[:, :V]
    return out


# revision 24
# speedup vs baseline: 1.0895x; 1.0895x over previous
"""GPT forward pass on 8 Trainium2 NeuronCores, optimized for wire traffic.

DP2 x TP4 compute: cores 0-3 batch 0, cores 4-7 batch 1. Within a group:
heads 16->4/core, FFN 4096->1024/core, vocab 50257->12800/core (padded).
The axon tunnel to the devices is ~40 MB/s, so wall time is dominated by
host<->device bytes, not compute. Wire-traffic reductions vs the plain
DP2xTP4 layout:
  - Trunk weights are 10-bit quantized (per-input-channel f32 scales,
    offset-binary hi-byte plane + 2-bit los packed 4/byte), unpacked and
    dequantized to bf16 on device with vector bit ops + fused activation
    scale/bias. Weight quant error adds ~7e-3 to the logits.
  - No weight is duplicated across the two DP groups: each core uploads
    half of its TP-rank's weights and a pair AllGather ({r, r+4})
    reconstructs the full shard on device.
  - lm_head weights are int8 with per-(input-channel, 512-vocab-block)
    scales, also pair-split + AllGathered, dequantized to bf16 on device.
  - x0 embeddings are bf16 quarter-shards AllGathered within each DP group.
  - The causal mask is generated on device (gpsimd affine_select).
  - Logits are returned as int8 with per-(token, 512-vocab-block) f32
    scales (the f32->int8 copy rounds to nearest on HW) and dequantized
    on host. This also halves the donated zero output buffer upload.
Compute structure is unchanged from the bf16 baseline: residual stream
transposed [D, S] fp32 in SBUF; bf16 matmuls, fp32 PSUM; LN scale/bias +
attention scale folded into weights host-side; softmax on transposed
scores without max-subtraction; two bf16 AllReduces per layer over
4-rank groups.
"""

import contextlib
import os

import numpy as np
import ml_dtypes

# Each run_bass_kernel_spmd call re-jits a fresh closure (guaranteed XLA
# in-memory cache miss); the persistent cache turns that ~1.7s recompile
# into a ~0.2s disk hit. Purely a compile cache - no effect on numerics.
try:
    import jax

    os.makedirs("/tmp/jaxcache", exist_ok=True)
    jax.config.update("jax_compilation_cache_dir", "/tmp/jaxcache")
    jax.config.update("jax_persistent_cache_min_compile_time_secs", 0)
    jax.config.update("jax_persistent_cache_min_entry_size_bytes", 0)
except Exception:
    pass

import concourse.bacc as bacc
import concourse.tile as tile
from concourse import mybir
from concourse.bass_utils import run_bass_kernel_spmd

BF = mybir.dt.bfloat16
F32 = mybir.dt.float32
I8 = mybir.dt.int8
U8 = mybir.dt.uint8
NPBF = ml_dtypes.bfloat16
AF = mybir.ActivationFunctionType
ALU = mybir.AluOpType

B, S, D, H, L, V = 2, 1024, 1024, 16, 8, 50257
HD, FF, EPS = 64, 4096, 1e-5
TP, HPC, FFC, VP, NC = 4, 4, 1024, 12800, 8
KC, TT, TS = 8, 2, 512
NT = VP // TS  # 25 vocab tiles per core

PAIRS = [[0, 4], [1, 5], [2, 6], [3, 7]]
QUADS = [[0, 1, 2, 3], [4, 5, 6, 7]]


def _pmajor(wt):
    """[K_contract, N] -> [128, K//128, N] partition-major."""
    k, n = wt.shape
    return np.ascontiguousarray(wt.reshape(k // 128, 128, n).transpose(1, 0, 2))


def _pack10(wt):
    """[K, M] f32 -> ([128, K//128, M + M//4] u8 packed uint10, [128, K//128]
    f32 scales). Values stored offset-binary: hi byte plane then lo 2-bit
    pairs packed 4/byte; w = (4*hi + lo - 512) * s with s per input chan."""
    k, m = wt.shape
    s = np.maximum(np.abs(wt).max(1) / 511.0, 1e-30)
    q = np.clip(np.rint(wt / s[:, None]), -511, 511).astype(np.int32)
    qu = (q + 512).astype(np.uint16)
    hi = (qu >> 2).astype(np.uint8)
    lo = (qu & 3).astype(np.uint8)
    lb = (lo[:, 0::4] | (lo[:, 1::4] << 2) | (lo[:, 2::4] << 4)
          | (lo[:, 3::4] << 6))
    pk = np.concatenate([hi, lb], 1)
    return _pmajor(pk), s.reshape(k // 128, 128).T.astype(np.float32)


def build_nc(n_layers=L):
    nc = bacc.Bacc("TRN2", target_bir_lowering=False, debug=False,
                   num_devices=NC)
    Lc = n_layers

    # ---- params: halves of pair-shared tensors, full small tensors ----
    # Trunk weights are 10-bit packed: per input channel, uint10 offset-
    # binary values as [hi byte | packed lo 2-bit pairs] along the free dim.
    x0q_d = nc.declare_dram_parameter("x0q", [32, KC, S], BF, isOutput=False)
    qkvwh_d = nc.declare_dram_parameter("qkvwh", [Lc * 64, KC, 960], U8, isOutput=False)
    projwh_d = nc.declare_dram_parameter("projwh", [Lc * 64, 2, 1280], U8, isOutput=False)
    fc1wh_d = nc.declare_dram_parameter("fc1wh", [Lc * 64, KC, 1280], U8, isOutput=False)
    fc2wh_d = nc.declare_dram_parameter("fc2wh", [Lc * 64, KC, 1280], U8, isOutput=False)
    qsc_d = nc.declare_dram_parameter("qsc", [128, Lc * KC], F32, isOutput=False)
    psc_d = nc.declare_dram_parameter("psc", [128, Lc * 2], F32, isOutput=False)
    f1sc_d = nc.declare_dram_parameter("f1sc", [128, Lc * KC], F32, isOutput=False)
    f2sc_d = nc.declare_dram_parameter("f2sc", [128, Lc * KC], F32, isOutput=False)
    headw8h_d = nc.declare_dram_parameter("headw8h", [64, KC, VP], I8, isOutput=False)
    headsc_d = nc.declare_dram_parameter("headsc", [128, KC * NT], F32, isOutput=False)
    qkvb_d = nc.declare_dram_parameter("qkvb", [Lc, 128, 6], F32, isOutput=False)
    projb_d = nc.declare_dram_parameter("projb", [Lc, 128, KC], F32, isOutput=False)
    fc1b_d = nc.declare_dram_parameter("fc1b", [Lc, 128, KC], F32, isOutput=False)
    fc2b_d = nc.declare_dram_parameter("fc2b", [Lc, 128, KC], F32, isOutput=False)
    headb_d = nc.declare_dram_parameter("headb", [1, VP], BF, isOutput=False)
    id_d = nc.declare_dram_parameter("id64", [64, 64], BF, isOutput=False)
    out_d = nc.declare_dram_parameter("out", [S, VP], I8, isOutput=True)
    oscale_d = nc.declare_dram_parameter("oscale", [NT, S], F32, isOutput=True)

    # ---- internal DRAM: staged halves (collectives can't read IO) + fulls
    x0q_i = nc.dram_tensor("x0q_i", [32, KC, S], BF)
    qkvwh_i = nc.dram_tensor("qkvwh_i", [Lc * 64, KC, 960], U8)
    projwh_i = nc.dram_tensor("projwh_i", [Lc * 64, 2, 1280], U8)
    fc1wh_i = nc.dram_tensor("fc1wh_i", [Lc * 64, KC, 1280], U8)
    fc2wh_i = nc.dram_tensor("fc2wh_i", [Lc * 64, KC, 1280], U8)
    headw8h_i = nc.dram_tensor("headw8h_i", [64, KC, VP], I8)
    x0_f = nc.dram_tensor("x0_f", [128, KC, S], BF)
    qkvw_f = nc.dram_tensor("qkvw_f", [Lc * 128, KC, 960], U8)
    projw_f = nc.dram_tensor("projw_f", [Lc * 128, 2, 1280], U8)
    fc1w_f = nc.dram_tensor("fc1w_f", [Lc * 128, KC, 1280], U8)
    fc2w_f = nc.dram_tensor("fc2w_f", [Lc * 128, KC, 1280], U8)
    headw8_f = nc.dram_tensor("headw8_f", [128, KC, VP], I8)

    ar_in = [nc.dram_tensor(f"arin{i}", [128, KC, S], BF) for i in range(2 * Lc)]
    ar_out = [nc.dram_tensor(f"arout{i}", [128, KC, S], BF) for i in range(2 * Lc)]

    with tile.TileContext(nc, num_cores=NC) as tc, contextlib.ExitStack() as ctx:
        # ---- reconstruct pair/quad-shared tensors on device ----
        for src, dst in ((x0q_d, x0q_i), (qkvwh_d, qkvwh_i),
                         (projwh_d, projwh_i), (fc1wh_d, fc1wh_i),
                         (fc2wh_d, fc2wh_i), (headw8h_d, headw8h_i)):
            nc.sync.dma_start(out=dst[:], in_=src[:])
        nc.gpsimd.collective_compute(
            "AllGather", ALU.bypass, ins=[x0q_i.ap().opt()],
            outs=[x0_f.ap().opt()], replica_groups=QUADS)
        for src, dst in ((qkvwh_i, qkvw_f), (projwh_i, projw_f),
                         (fc1wh_i, fc1w_f), (fc2wh_i, fc2w_f),
                         (headw8h_i, headw8_f)):
            nc.gpsimd.collective_compute(
                "AllGather", ALU.bypass, ins=[src.ap().opt()],
                outs=[dst.ap().opt()], replica_groups=PAIRS)

        # ---- persistent pools (LN machinery, residual, outputs) ----
        consts = ctx.enter_context(tc.tile_pool(name="consts", bufs=1))
        xpool = ctx.enter_context(tc.tile_pool(name="x", bufs=1))
        zpool = ctx.enter_context(tc.tile_pool(name="z", bufs=1))
        rows = ctx.enter_context(tc.tile_pool(name="rows", bufs=2))
        qrows = ctx.enter_context(tc.tile_pool(name="qrows", bufs=2))
        bcast = ctx.enter_context(tc.tile_pool(name="bcast", bufs=2))
        sq_p = ctx.enter_context(tc.tile_pool(name="sq", bufs=2))
        outp = ctx.enter_context(tc.tile_pool(name="outs", bufs=2))
        ps_st = ctx.enter_context(tc.tile_pool(name="psst", bufs=2, space="PSUM"))
        ps_bc = ctx.enter_context(tc.tile_pool(name="psbc", bufs=1, space="PSUM"))

        mask_sb = consts.tile([128, 4, TS], BF, tag="mask")
        for j in range(4):
            # mask[p, j, s] = 1.0 if s >= j*128 + p else 0.0
            nc.gpsimd.memset(mask_sb[:, j, :], 1.0)
            nc.gpsimd.affine_select(
                out=mask_sb[:, j, :], in_=mask_sb[:, j, :],
                compare_op=ALU.is_ge, fill=0.0, base=-j * 128,
                pattern=[[1, TS]], channel_multiplier=-1)
        idt = consts.tile([128, 64], BF, tag="idt")
        nc.sync.dma_start(out=idt[0:64, :], in_=id_d[:])
        nc.sync.dma_start(out=idt[64:128, :], in_=id_d[:])
        ones = consts.tile([128, 128], BF, tag="ones")
        nc.vector.memset(ones[:], 1.0)
        hsc = consts.tile([128, KC * NT], F32, tag="hsc")
        nc.sync.dma_start(out=hsc[:], in_=headsc_d[:])
        # trunk weight scales + their -2048*s biases for 12-bit unpack
        wsc, wnb = {}, {}
        for nm, src, ncol in (("q", qsc_d, Lc * KC), ("p", psc_d, Lc * 2),
                              ("f1", f1sc_d, Lc * KC), ("f2", f2sc_d, Lc * KC)):
            sct = consts.tile([128, ncol], F32, tag=f"sc_{nm}")
            nc.sync.dma_start(out=sct[:], in_=src[:])
            nbt = consts.tile([128, ncol], F32, tag=f"nb_{nm}")
            nc.vector.tensor_scalar_mul(nbt[:], sct[:], -512.0)
            wsc[nm], wnb[nm] = sct, nbt

        x_sb = xpool.tile([128, KC, S], F32, tag="x")
        zx = zpool.tile([128, KC, S], BF, tag="z")  # bf16 x, normalized in place
        # x0 arrives bf16 via the quad AllGather; stage through zx into f32.
        nc.sync.dma_start(out=zx[:], in_=x0_f[:])
        for c in range(KC):
            nc.scalar.activation(out=x_sb[:, c, :], in_=zx[:, c, :], func=AF.Copy)

        def layer_norm():
            for c in range(KC):
                nc.scalar.activation(out=zx[:, c, :], in_=x_sb[:, c, :],
                                     func=AF.Copy)
            mu_b, rs_b = [None] * TT, [None] * TT
            for t in range(TT):
                sl = slice(t * TS, (t + 1) * TS)
                ps_s = ps_st.tile([1, TS], F32, tag="st")
                ps_q = ps_st.tile([1, TS], F32, tag="st")
                for c in range(KC):
                    sq = sq_p.tile([128, TS], BF, tag="sq")
                    nc.vector.tensor_mul(sq[:], zx[:, c, sl], zx[:, c, sl])
                    nc.tensor.matmul(ps_s[:], ones[:, 0:1], zx[:, c, sl],
                                     start=(c == 0), stop=(c == KC - 1))
                    nc.tensor.matmul(ps_q[:], ones[:, 0:1], sq[:],
                                     start=(c == 0), stop=(c == KC - 1))
                mu = rows.tile([1, TS], F32, tag="mu")
                nc.vector.tensor_scalar_mul(mu[:], ps_s[:], 1.0 / D)
                ms = rows.tile([1, TS], F32, tag="ms")
                nc.vector.tensor_mul(ms[:], mu[:], mu[:])
                ve = rows.tile([1, TS], F32, tag="ve")
                nc.vector.tensor_scalar(ve[:], ps_q[:], 1.0 / D, EPS,
                                        op0=ALU.mult, op1=ALU.add)
                nc.vector.tensor_sub(ve[:], ve[:], ms[:])
                sd = rows.tile([1, TS], F32, tag="sd")
                nc.scalar.activation(out=sd[:], in_=ve[:], func=AF.Sqrt)
                rs = rows.tile([1, TS], BF, tag="rs")
                with nc.allow_low_precision(reason="bf16 rstd row"):
                    nc.vector.reciprocal(rs[:], sd[:])
                mubf = rows.tile([1, TS], BF, tag="mubf")
                nc.vector.tensor_copy(mubf[:], mu[:])
                ps_mb = ps_bc.tile([128, TS], F32, tag="bc")
                nc.tensor.matmul(ps_mb[:], ones[0:1, :], mubf[:])
                mb = bcast.tile([128, TS], BF, tag="mb")
                nc.scalar.activation(out=mb[:], in_=ps_mb[:], func=AF.Copy)
                ps_rb = ps_bc.tile([128, TS], F32, tag="bc")
                nc.tensor.matmul(ps_rb[:], ones[0:1, :], rs[:])
                rb = bcast.tile([128, TS], BF, tag="rb")
                nc.scalar.activation(out=rb[:], in_=ps_rb[:], func=AF.Copy)
                mu_b[t], rs_b[t] = mb, rb
            for c in range(KC):
                for t in range(TT):
                    sl = slice(t * TS, (t + 1) * TS)
                    nc.vector.tensor_sub(zx[:, c, sl], zx[:, c, sl], mu_b[t][:])
                    nc.vector.tensor_mul(zx[:, c, sl], zx[:, c, sl], rs_b[t][:])

        # ---- trunk ----
        with contextlib.ExitStack() as tctx:
            wq = tctx.enter_context(tc.tile_pool(name="wq", bufs=1))
            wp = tctx.enter_context(tc.tile_pool(name="wp", bufs=1))
            w1 = tctx.enter_context(tc.tile_pool(name="w1", bufs=1))
            w2 = tctx.enter_context(tc.tile_pool(name="w2", bufs=1))
            bpool = tctx.enter_context(tc.tile_pool(name="bias", bufs=2))
            qkvo = tctx.enter_context(tc.tile_pool(name="qkvo", bufs=1))
            probs = tctx.enter_context(tc.tile_pool(name="probs", bufs=1))
            attn = tctx.enter_context(tc.tile_pool(name="attn", bufs=2))
            apool = tctx.enter_context(tc.tile_pool(name="act", bufs=1))
            stage = tctx.enter_context(tc.tile_pool(name="stage", bufs=3))
            pkst = tctx.enter_context(tc.tile_pool(name="pkst", bufs=2))
            lost = tctx.enter_context(tc.tile_pool(name="lost", bufs=1))
            accst = tctx.enter_context(tc.tile_pool(name="accst", bufs=1))
            ps_mm = tctx.enter_context(
                tc.tile_pool(name="psmm", bufs=2, space="PSUM"))
            ps_sc = tctx.enter_context(
                tc.tile_pool(name="pssc", bufs=2, space="PSUM"))
            ps_ao = tctx.enter_context(
                tc.tile_pool(name="psao", bufs=1, space="PSUM"))

            def load_packed(dst_bf, pk_f, lsl, kc_n, m, nm, l):
                """10-bit unpack: dst = (4*hi + lo - 512) * s, per in-chan."""
                sct, nbt = wsc[nm], wnb[nm]
                q = m // 4
                for kc in range(kc_n):
                    co = l * kc_n + kc
                    pk = pkst.tile([128, 1280], U8, tag="pk")
                    nc.sync.dma_start(out=pk[:, 0:m + q],
                                      in_=pk_f[lsl, kc, :])
                    acc = accst.tile([128, m], F32, tag="acc")
                    nc.vector.tensor_scalar_mul(acc[:], pk[:, 0:m], 4.0)
                    for j in range(4):
                        lo = lost.tile([128, q], U8, tag=f"lo{j}")
                        if j == 0:
                            nc.vector.tensor_scalar(
                                lo[:], pk[:, m:m + q], 3, None,
                                op0=ALU.bitwise_and)
                        elif j == 3:
                            nc.vector.tensor_scalar(
                                lo[:], pk[:, m:m + q], 6, None,
                                op0=ALU.logical_shift_right)
                        else:
                            nc.vector.tensor_scalar(
                                lo[:], pk[:, m:m + q], 2 * j, 3,
                                op0=ALU.logical_shift_right,
                                op1=ALU.bitwise_and)
                        nc.vector.tensor_add(acc[:, j:m:4], acc[:, j:m:4],
                                             lo[:])
                    nc.scalar.activation(out=dst_bf[:, kc, :], in_=acc[:],
                                         func=AF.Identity,
                                         scale=sct[:, co:co + 1],
                                         bias=nbt[:, co:co + 1])

            def mm_block(wt, bias_tile, rhs_sb, n_out, kc_n, out_cb, func):
                for mt in range(n_out // 128):
                    msl = slice(mt * 128, (mt + 1) * 128)
                    for t in range(TT):
                        sl = slice(t * TS, (t + 1) * TS)
                        ps = ps_mm.tile([128, TS], F32, tag="mm")
                        for kc in range(kc_n):
                            nc.tensor.matmul(ps[:], wt[:, kc, msl],
                                             rhs_sb(kc, sl), start=(kc == 0),
                                             stop=(kc == kc_n - 1))
                        out_cb(mt, t, ps, bias_tile[:, mt:mt + 1], func)

            def evict(dst_ap):
                def cb(mt, t, ps, bias, func):
                    nc.scalar.activation(out=dst_ap(mt, t), in_=ps[:],
                                         func=func, bias=bias)
                return cb

            def evict_ar(ar_buf):
                def cb(mt, t, ps, bias, func):
                    st = stage.tile([128, TS], BF, tag="arst")
                    nc.scalar.activation(out=st[:], in_=ps[:], func=func,
                                         bias=bias)
                    nc.sync.dma_start(out=ar_buf[:, mt, t * TS:(t + 1) * TS],
                                      in_=st[:])
                return cb

            def allreduce_residual(li):
                nc.gpsimd.collective_compute(
                    "AllReduce", ALU.add,
                    ins=[ar_in[li].ap().opt()], outs=[ar_out[li].ap().opt()],
                    replica_groups=QUADS)
                for c in range(KC):
                    st = stage.tile([128, S], BF, tag="arld")
                    nc.sync.dma_start(out=st[:], in_=ar_out[li][:, c, :])
                    nc.vector.tensor_add(x_sb[:, c, :], x_sb[:, c, :], st[:])

            for l in range(Lc):
                lsl = slice(l * 128, (l + 1) * 128)
                layer_norm()
                qw = wq.tile([128, KC, 768], BF, tag="qkvw")
                load_packed(qw, qkvw_f, lsl, KC, 768, "q", l)
                qb = bpool.tile([128, 6], F32, tag="qkvb")
                nc.sync.dma_start(out=qb[:], in_=qkvb_d[l])
                qkv = qkvo.tile([128, 6, S], BF, tag="qkv")
                mm_block(qw, qb, lambda kc, sl: zx[:, kc, sl], 768, KC,
                         evict(lambda mt, t: qkv[:, mt, t * TS:(t + 1) * TS]),
                         AF.Identity)

                aon = attn.tile([128, 2, S], BF, tag="aon")
                for h in range(HPC):
                    hb = (h % 2) * 64
                    hsl = slice(hb, hb + 64)
                    vt = attn.tile([128, KC, 64], BF, tag="vt")
                    for kt in range(KC):
                        pvt = ps_mm.tile([128, 64], BF, tag="mm")
                        nc.tensor.transpose(
                            pvt[:], qkv[hsl, 4 + h // 2, kt * 128:(kt + 1) * 128],
                            idt[hsl, :])
                        nc.scalar.activation(out=vt[:, kt, :], in_=pvt[:],
                                             func=AF.Copy)
                    for t in range(TT):
                        sl = slice(t * TS, (t + 1) * TS)
                        nkt = 4 * (t + 1)
                        pb = probs.tile([128, KC, TS], BF, tag="probs")
                        ps_d = ps_st.tile([1, TS], F32, tag="st")
                        for kt in range(nkt):
                            psc = ps_sc.tile([128, TS], F32, tag="sc")
                            nc.tensor.matmul(
                                psc[:], qkv[hsl, 2 + h // 2,
                                            kt * 128:(kt + 1) * 128],
                                qkv[hsl, h // 2, sl])
                            nc.scalar.activation(out=pb[:, kt, :], in_=psc[:],
                                                 func=AF.Exp)
                            moff = kt - t * 4
                            if moff >= 0:
                                nc.vector.tensor_mul(
                                    pb[:, kt, :], pb[:, kt, :],
                                    mask_sb[:, moff, :])
                            nc.tensor.matmul(ps_d[:], ones[:, 0:1],
                                             pb[:, kt, :], start=(kt == 0),
                                             stop=(kt == nkt - 1))
                        rr = rows.tile([1, TS], BF, tag="rr")
                        with nc.allow_low_precision(reason="bf16 softmax recip"):
                            nc.vector.reciprocal(rr[:], ps_d[:])
                        ps_rb = ps_bc.tile([128, TS], F32, tag="bc")
                        nc.tensor.matmul(ps_rb[hsl, :], ones[0:1, hsl], rr[:])
                        rb = bcast.tile([128, TS], BF, tag="arb")
                        nc.scalar.activation(out=rb[hsl, :], in_=ps_rb[hsl, :],
                                             func=AF.Copy)
                        pao = ps_ao.tile([128, TS], F32, tag="ao")
                        for kt in range(nkt):
                            nc.tensor.matmul(pao[hsl, :], vt[:, kt, :],
                                             pb[:, kt, :], start=(kt == 0),
                                             stop=(kt == nkt - 1))
                        nc.vector.tensor_mul(aon[hsl, h // 2, sl],
                                             pao[hsl, :], rb[hsl, :])

                pw = wp.tile([128, 2, D], BF, tag="projw")
                load_packed(pw, projw_f, lsl, 2, D, "p", l)
                pbias = bpool.tile([128, KC], F32, tag="projb")
                nc.sync.dma_start(out=pbias[:], in_=projb_d[l])
                mm_block(pw, pbias, lambda kc, sl: aon[:, kc, sl], D, 2,
                         evict_ar(ar_in[2 * l]), AF.Identity)
                allreduce_residual(2 * l)

                layer_norm()
                w1t = w1.tile([128, KC, FFC], BF, tag="fc1w")
                load_packed(w1t, fc1w_f, lsl, KC, FFC, "f1", l)
                b1 = bpool.tile([128, KC], F32, tag="fc1b")
                nc.sync.dma_start(out=b1[:], in_=fc1b_d[l])
                a_sb = apool.tile([128, KC, S], BF, tag="a")
                mm_block(w1t, b1, lambda kc, sl: zx[:, kc, sl], FFC, KC,
                         evict(lambda mt, t: a_sb[:, mt, t * TS:(t + 1) * TS]),
                         AF.Relu)
                w2t = w2.tile([128, KC, D], BF, tag="fc2w")
                load_packed(w2t, fc2w_f, lsl, KC, D, "f2", l)
                b2 = bpool.tile([128, KC], F32, tag="fc2b")
                nc.sync.dma_start(out=b2[:], in_=fc2b_d[l])
                mm_block(w2t, b2, lambda kc, sl: a_sb[:, kc, sl], D, KC,
                         evict_ar(ar_in[2 * l + 1]), AF.Identity)
                allreduce_residual(2 * l + 1)

            layer_norm()

        # ---- lm_head (trunk pools closed; zx holds final LN output) ----
        with contextlib.ExitStack() as lctx:
            hwp = lctx.enter_context(tc.tile_pool(name="hw", bufs=3))
            hw8p = lctx.enter_context(tc.tile_pool(name="hw8", bufs=2))
            ps_lm = lctx.enter_context(
                tc.tile_pool(name="pslm", bufs=3, space="PSUM"))
            hb_row = hwp.tile([1, VP], BF, tag="hbrow")
            nc.sync.dma_start(out=hb_row[:], in_=headb_d[:])
            hw_t = {}
            for nt in range(NT):
                t8 = hw8p.tile([128, KC, TS], I8, tag="hw8")
                nc.sync.dma_start(out=t8[:],
                                  in_=headw8_f[:, :, nt * TS:(nt + 1) * TS])
                t0 = hwp.tile([128, KC, TS], BF, tag="hw")
                for kc in range(KC):
                    nc.scalar.activation(
                        out=t0[:, kc, :], in_=t8[:, kc, :], func=AF.Copy,
                        scale=hsc[:, kc * NT + nt:kc * NT + nt + 1])
                hw_t[nt] = t0
                if nt % 2 == 1 or nt == NT - 1:
                    nts = [nt - 1, nt] if nt % 2 == 1 else [nt]
                    bsts = {}
                    for n2 in nts:
                        psb = ps_bc.tile([128, TS], F32, tag="bc")
                        nc.tensor.matmul(psb[:], ones[0:1, :],
                                         hb_row[:, n2 * TS:(n2 + 1) * TS])
                        bst = bcast.tile([128, TS], F32, tag="hb")
                        nc.scalar.activation(out=bst[:], in_=psb[:],
                                             func=AF.Copy)
                        bsts[n2] = bst
                    for mt in range(KC):
                        pss = {n2: ps_lm.tile([128, TS], F32, tag="lm",
                                              name=f"pslm{n2 % 2}")
                               for n2 in nts}
                        for kc in range(KC):
                            for n2 in nts:
                                nc.tensor.matmul(
                                    pss[n2][:],
                                    zx[:, kc, mt * 128:(mt + 1) * 128],
                                    hw_t[n2][:, kc, :], start=(kc == 0),
                                    stop=(kc == KC - 1))
                        for n2 in nts:
                            of = outp.tile([128, TS], F32, tag="of")
                            nc.vector.tensor_add(of[:], pss[n2][:],
                                                 bsts[n2][:])
                            mx = qrows.tile([128, 1], F32, tag="mx")
                            nc.vector.reduce_max(
                                out=mx[:], in_=of[:],
                                axis=mybir.AxisListType.X,
                                apply_absolute_value=True)
                            sc = qrows.tile([128, 1], F32, tag="sc")
                            nc.vector.tensor_scalar(
                                sc[:], mx[:], 1.0 / 126.0, 1e-30,
                                op0=ALU.mult, op1=ALU.max)
                            iv = qrows.tile([128, 1], F32, tag="iv")
                            nc.vector.reciprocal(iv[:], sc[:])
                            nc.vector.tensor_scalar_mul(of[:], of[:], iv[:])
                            qt = outp.tile([128, TS], I8, tag="qt")
                            nc.scalar.activation(out=qt[:], in_=of[:],
                                                 func=AF.Copy)
                            nc.sync.dma_start(
                                out=out_d[mt * 128:(mt + 1) * 128,
                                          n2 * TS:(n2 + 1) * TS], in_=qt[:])
                            nc.sync.dma_start(
                                out=oscale_d[n2, mt * 128:(mt + 1) * 128],
                                in_=sc[:])
                    for n2 in nts:
                        del hw_t[n2]
    nc.compile()
    return nc


def prep_inputs(inputs, n_layers=L):
    i = {k: np.asarray(v) for k, v in inputs.items()}
    idx, tok_emb, pos_emb = i["idx"], i["tok_emb"], i["pos_emb"]
    hw_pad = np.zeros((TP * VP, D), np.float32)
    hw_pad[:V] = i["head_w"] * i["lnf_s"][None, :]
    hb_pad = np.zeros((TP * VP,), np.float32)
    hb_pad[:V] = i["head_b"] + i["head_w"] @ i["lnf_b"]
    id64 = np.eye(64, dtype=NPBF)

    # per-group x0 quarter shards (bf16, partition-major flat quarters)
    x0q = {}
    for g in range(B):
        x0 = (tok_emb[idx[g]] + pos_emb).astype(np.float32)  # [S, D]
        pm = np.ascontiguousarray(
            x0.T.reshape(KC, 128, S).transpose(1, 0, 2)).astype(NPBF)
        x0q[g] = pm.reshape(4, 32, KC, S)

    # per-rank weights (computed once, split into pair halves)
    rank = []
    for r in range(TP):
        qkvw = np.empty((n_layers, 128, KC, 960), np.uint8)
        qsc = np.empty((n_layers, 128, KC), np.float32)
        qkvb = np.empty((n_layers, 128, 6), np.float32)
        projw = np.empty((n_layers, 128, 2, 1280), np.uint8)
        psc = np.empty((n_layers, 128, 2), np.float32)
        projb = np.empty((n_layers, 128, KC), np.float32)
        fc1w = np.empty((n_layers, 128, KC, 1280), np.uint8)
        f1sc = np.empty((n_layers, 128, KC), np.float32)
        fc1b = np.empty((n_layers, 128, KC), np.float32)
        fc2w = np.empty((n_layers, 128, KC, 1280), np.uint8)
        f2sc = np.empty((n_layers, 128, KC), np.float32)
        fc2b = np.empty((n_layers, 128, KC), np.float32)
        for l in range(n_layers):
            qw = i["qkv_w"][l]  # [3D, D]; row h*192 + {q:0,k:64,v:128} + hd
            blk = {"q": [], "k": [], "v": []}
            for j in range(HPC):
                h = r * HPC + j
                blk["q"].append(qw[h * 192:h * 192 + 64])
                blk["k"].append(qw[h * 192 + 64:h * 192 + 128])
                blk["v"].append(qw[h * 192 + 128:h * 192 + 192])
            W = np.concatenate(blk["q"] + blk["k"] + blk["v"], 0)  # [768, D]
            beff = W @ i["ln1_b"][l]
            Wp = W * i["ln1_s"][l][None, :]
            Wp[:256] *= HD ** -0.5
            beff[:256] *= HD ** -0.5
            qkvw[l], qsc[l] = _pack10(Wp.T)
            qkvb[l] = beff.reshape(6, 128).T
            projw[l], psc[l] = _pack10(
                i["proj_w"][l][:, r * 256:(r + 1) * 256].T)
            projb[l] = (i["proj_b"][l] / TP).reshape(KC, 128).T
            W1 = i["fc1_w"][l][r * FFC:(r + 1) * FFC]  # [FFC, D]
            fc1b[l] = (i["fc1_b"][l][r * FFC:(r + 1) * FFC]
                       + W1 @ i["ln2_b"][l]).reshape(KC, 128).T
            fc1w[l], f1sc[l] = _pack10((W1 * i["ln2_s"][l][None, :]).T)
            fc2w[l], f2sc[l] = _pack10(
                i["fc2_w"][l][:, r * FFC:(r + 1) * FFC].T)
            fc2b[l] = (i["fc2_b"][l] / TP).reshape(KC, 128).T

        # int8 head weights, scale per (input channel, 512-vocab block)
        WT = np.ascontiguousarray(hw_pad[r * VP:(r + 1) * VP].T)  # [D, VP]
        scs = np.maximum(
            np.abs(WT.reshape(D, NT, TS)).max(2) / 127.0, 1e-30)  # [D, NT]
        q8 = np.clip(np.rint(WT.reshape(D, NT, TS) / scs[:, :, None]),
                     -127, 127).astype(np.int8).reshape(D, VP)
        headw8 = _pmajor(q8)  # [128, KC, VP] int8
        headsc = np.ascontiguousarray(
            scs.reshape(KC, 128, NT).transpose(1, 0, 2)
        ).reshape(128, KC * NT).astype(np.float32)

        halves = {}
        for name, arr in (("qkvwh", qkvw), ("projwh", projw),
                          ("fc1wh", fc1w), ("fc2wh", fc2w)):
            flat = arr.reshape(n_layers * 128, *arr.shape[2:])
            halves[name] = (np.ascontiguousarray(flat[:n_layers * 64]),
                            np.ascontiguousarray(flat[n_layers * 64:]))
        halves["headw8h"] = (np.ascontiguousarray(headw8[:64]),
                             np.ascontiguousarray(headw8[64:]))

        def sc_cols(a):  # [Lc,128,g] -> [128, Lc*g] with column l*g + kc
            return np.ascontiguousarray(
                a.transpose(1, 0, 2).reshape(128, -1))
        rank.append(dict(
            halves=halves, headsc=headsc, qkvb=qkvb, projb=projb,
            fc1b=fc1b, fc2b=fc2b, qsc=sc_cols(qsc), psc=sc_cols(psc),
            f1sc=sc_cols(f1sc), f2sc=sc_cols(f2sc),
            headb=hb_pad[None, r * VP:(r + 1) * VP].astype(NPBF)))

    in_maps = []
    for core in range(NC):
        g, r = divmod(core, TP)
        rd = rank[r]
        m = {
            "x0q": np.ascontiguousarray(x0q[g][r]),
            "id64": id64,
            "headsc": rd["headsc"], "headb": rd["headb"],
            "qkvb": rd["qkvb"], "projb": rd["projb"],
            "fc1b": rd["fc1b"], "fc2b": rd["fc2b"],
            "qsc": rd["qsc"], "psc": rd["psc"],
            "f1sc": rd["f1sc"], "f2sc": rd["f2sc"],
        }
        for name in ("qkvwh", "projwh", "fc1wh", "fc2wh", "headw8h"):
            m[name] = rd["halves"][name][g]
        in_maps.append(m)
    return in_maps


_NC_CACHE = {}


def kernel(**inputs):
    if L not in _NC_CACHE:
        _NC_CACHE[L] = build_nc(L)
    nc = _NC_CACHE[L]
    in_maps = prep_inputs(inputs)
    res = run_bass_kernel_spmd(nc, in_maps, core_ids=list(range(NC)))
    return assemble_output(res)


def assemble_output(res):
    out = np.empty((B, S, V), np.float32)
    for g in range(B):
        parts = []
        for r in range(TP):
            rr = res.results[g * TP + r]
            q = rr["out"].astype(np.float32).reshape(S, NT, TS)
            q *= rr["oscale"].T[:, :, None]  # [S, NT, 1]
            parts.append(q.reshape(S, VP))
        out[g] = np.concatenate(parts, axis=1)[:, :V]
    return out
